# revision 4
# baseline (speedup 1.0000x reference)
"""BasketEmbedding Trainium2 kernel (Bass/Tile, 8 NeuronCores, SPMD).

Reference semantics (B=1024, S=50, M=20, H=128, table 100001x128 f32,
padding_idx = 100000 whose row is zero):

    emb    = table[item_ids]                             # [B,S,M,H]
    summed = sum over m < basket_lens[b,s] of emb        # [B,S,H]
    pooled = summed / basket_lens                        # mean pool
    out    = where(s < seq_lens[b], pooled, 100000.0)    # [B,S,H]

Strategy: data-parallel over batch — each of the 8 cores handles 128
batches (6400 baskets). The dynamic-DMA ucode on this runtime consumes
exactly one offset per contiguous output run per partition, so one
indirect DMA can gather at most 128 table rows (one per partition) and
its ~1.5 us fixed SWDGE cost dominates. To minimize instruction count,
the host assigns baskets to (partition, group) slots sorted by
"effective length" (1 for sequence-padded baskets — their output is a
constant and their single slot points at the zero padding row, else
basket_len), so group g only needs L_g = max length in that group
gather instructions (~300 total instead of 50*20). On device: item
slots past each basket's length are remapped to the zero padding row,
a DVE tensor_reduce sums each group's gathered rows, and a fused
tensor_scalar applies 1/len and the sequence-validity mask. The host
maps output rows back to their natural (b, s) positions (pure layout).
"""

import numpy as np

import concourse.bass as bass
import concourse.mybir as mybir
import concourse.tile as tile
from concourse.bass_utils import run_bass_kernel_spmd

N_CORES = 8


def _split_multi_waits(nc):
    """Walrus on this stack rejects >1 sync-wait command per instruction
    ("Too many sync wait commands", CoreV3GenImpl setupSyncWait). Tile
    freely attaches several SyncWaits to one instruction, so hoist all
    but the last wait of each instruction onto same-engine NoOps
    inserted directly before it — identical sequencer semantics.
    """
    fn = nc.m.functions[0]
    for bb in fn.blocks:
        insts = bb.instructions
        if not any(i.sync_info and i.sync_info.on_wait
                   and len(i.sync_info.on_wait) > 1 for i in insts):
            continue
        new_list = []
        for inst in insts:
            si = inst.sync_info
            if si is not None and si.on_wait and len(si.on_wait) > 1:
                waits = list(si.on_wait)
                for k, w in enumerate(waits[:-1]):
                    nop = mybir.InstNoOp(name=f"{inst.name}-w{k}", ins=[],
                                         outs=[])
                    nop.engine = inst.engine
                    nop.sync_info = mybir.SyncInfo(on_wait=[w], on_update=[])
                    new_list.append(nop)
                inst.sync_info = mybir.SyncInfo(
                    on_wait=[waits[-1]],
                    on_update=list(si.on_update) if si.on_update else [])
            new_list.append(inst)
        bb.instructions = new_list


P = 128        # SBUF partitions = baskets per group; batches per core
S = 50         # sequence positions; also groups per core (6400/128)
M = 20         # max items per basket
H = 128        # hidden size
NROWS = 100001
PAD_ID = 100000
PAD_VAL = 100000.0

F32 = mybir.dt.float32
BF16 = mybir.dt.bfloat16
I32 = mybir.dt.int32
OP = mybir.AluOpType


def build_nc(lprofile, ng, m=M, h=H, nrows=NROWS, pad_id=PAD_ID,
             pad_val=PAD_VAL, gather_bufs=20):
    """Build the per-core program. lprofile[g] = number of item slots to
    gather for group g (= max effective basket length in the group)."""
    nc = bass.Bass(num_swdge_queues=2)

    table = nc.dram_tensor("table", [nrows, h], BF16, kind="ExternalInput").ap()
    ids = nc.dram_tensor("ids", [P, ng * m], I32, kind="ExternalInput").ap()
    lens = nc.dram_tensor("lens", [P, ng], I32, kind="ExternalInput").ap()
    sidx = nc.dram_tensor("sidx", [P, ng], I32, kind="ExternalInput").ap()
    slen = nc.dram_tensor("slen", [P, ng], I32, kind="ExternalInput").ap()
    out = nc.dram_tensor("out", [P, ng, h], F32, kind="ExternalOutput").ap()

    with tile.TileContext(nc) as tc:
        with (
            tc.tile_pool(name="const", bufs=1) as cpool,
            tc.tile_pool(name="gather", bufs=gather_bufs) as gpool,
            tc.tile_pool(name="acc", bufs=8) as apool,
            tc.tile_pool(name="fin", bufs=8) as fpool,
        ):
            # Tile dependency tracking is tile-granular, so group 0's inputs
            # get physically separate tiles — its gathers then wait only on
            # three tiny ops instead of the full-tensor mask chain.
            ids0_t = cpool.tile([P, m], I32, tag="ids0")
            nc.sync.dma_start(ids0_t[:], ids[:, 0:m])
            lens0_t = cpool.tile([P, 1], I32, tag="lens0")
            nc.sync.dma_start(lens0_t[:], lens[:, 0:1])
            ids_t = cpool.tile([P, ng * m], I32, tag="ids")
            nc.sync.dma_start(ids_t[:, m:], ids[:, m:])
            lens_t = cpool.tile([P, ng], I32, tag="lens")
            nc.sync.dma_start(lens_t[:], lens)
            sidx_t = cpool.tile([P, ng], I32, tag="sidx")
            nc.sync.dma_start(sidx_t[:], sidx)
            slen_t = cpool.tile([P, ng], I32, tag="slen")
            nc.sync.dma_start(slen_t[:], slen)

            # miota[p, g*m + j] = j  (item slot index within basket)
            miota = cpool.tile([P, ng * m], I32, tag="miota")
            nc.gpsimd.iota(miota[:], pattern=[[0, ng], [1, m]], base=0,
                           channel_multiplier=0)

            # Masked ids: slots at/past the basket length -> padding row
            # (whose embedding is all zeros):  id' = max(id, (j>=len)*pad).
            # Computed in two chunks so the first gather group's columns
            # are ready without waiting for the whole id tensor.
            pm0 = cpool.tile([P, m], I32, tag="pm0")
            mid0_t = cpool.tile([P, m], I32, tag="mid0")
            nc.vector.tensor_tensor(
                out=pm0[:], in0=miota[:, 0:m],
                in1=lens0_t[:].to_broadcast([P, m]), op=OP.is_ge)
            nc.vector.tensor_scalar(
                out=pm0[:], in0=pm0[:], scalar1=pad_id, scalar2=None,
                op0=OP.mult)
            nc.vector.tensor_tensor(
                out=mid0_t[:], in0=ids0_t[:], in1=pm0[:], op=OP.max)

            pm = cpool.tile([P, ng * m], I32, tag="pm")
            mid_t = cpool.tile([P, ng * m], I32, tag="mid")
            nc.vector.tensor_tensor(
                out=pm[:, m:], in0=miota[:, m:],
                in1=lens_t[:, 1:ng].broadcast_to([P, ng - 1, m]), op=OP.is_ge)
            nc.vector.tensor_scalar(
                out=pm[:, m:], in0=pm[:, m:], scalar1=pad_id, scalar2=None,
                op0=OP.mult)
            nc.vector.tensor_tensor(
                out=mid_t[:, m:], in0=ids_t[:, m:], in1=pm[:, m:], op=OP.max)

            # Fused epilogue coefficients per slot:
            #   valid  (s <  seq_len): out = acc * (1/len) + 0
            #   padded (s >= seq_len): out = acc * 0       + pad_val
            lens_f = cpool.tile([P, ng], F32, tag="lensf")
            nc.vector.tensor_copy(out=lens_f[:], in_=lens_t[:])
            recip = cpool.tile([P, ng], F32, tag="recip")
            nc.vector.reciprocal(recip[:], lens_f[:])
            smask = cpool.tile([P, ng], F32, tag="smask")
            nc.vector.tensor_tensor(
                out=smask[:], in0=sidx_t[:], in1=slen_t[:], op=OP.is_lt)
            scale = cpool.tile([P, ng], F32, tag="scale")
            nc.vector.tensor_tensor(
                out=scale[:], in0=smask[:], in1=recip[:], op=OP.mult)
            offs = cpool.tile([P, ng], F32, tag="offs")
            nc.vector.tensor_scalar(
                out=offs[:], in0=smask[:], scalar1=-pad_val, scalar2=pad_val,
                op0=OP.mult, op1=OP.add)

            # Emit the all-padded (no-gather) groups first so their copies
            # and stores overlap the gather phase instead of trailing it.
            gorder = ([g for g in range(ng) if lprofile[g] == 0]
                      + [g for g in range(ng) if lprofile[g] > 0])
            for g in gorder:
                lg = int(lprofile[g])
                ft = fpool.tile([P, h], F32, tag="ft")
                if lg == 0:
                    # Group of sequence-padded baskets only: output is the
                    # constant pad vector; no gather needed.
                    nc.vector.tensor_copy(
                        out=ft[:], in_=offs[:, g:g + 1].to_broadcast([P, h]))
                    nc.sync.dma_start(out[:, g, :], ft[:])
                else:
                    gt = gpool.tile([P, lg * h], BF16, tag="gt")
                    # One [P,1]-offset indirect DMA per item slot: the ucode
                    # consumes one offset per contiguous output run/partition.
                    midsrc = mid0_t if g == 0 else mid_t
                    for j in range(lg):
                        gi = nc.gpsimd.indirect_dma_start(
                            out=gt[:, j * h:(j + 1) * h], out_offset=None,
                            in_=table,
                            in_offset=bass.IndirectOffsetOnAxis(
                                ap=midsrc[:, g * m + j:g * m + j + 1], axis=0),
                        )
                        if (g + j) % 2:
                            gi.ins.queue = "qPoolDynamic1"

                    acc = apool.tile([P, h], F32, tag="acc")
                    nc.vector.tensor_reduce(
                        out=acc[:],
                        in_=gt[:].rearrange("p (m h) -> p h m", m=lg),
                        axis=mybir.AxisListType.X, op=OP.add)
                    nc.vector.tensor_scalar(
                        out=ft[:], in0=acc[:],
                        scalar1=scale[:, g:g + 1], scalar2=offs[:, g:g + 1],
                        op0=OP.mult, op1=OP.add)
                    nc.sync.dma_start(out[:, g, :], ft[:])

    _split_multi_waits(nc)
    return nc


_NC_CACHE = {}


def kernel(table, item_ids, basket_lens, seq_lens):
    import ml_dtypes
    table = np.ascontiguousarray(
        np.asarray(table, dtype=np.float32).astype(ml_dtypes.bfloat16))
    ids = np.ascontiguousarray(np.asarray(item_ids)).astype(np.int32)
    lens = np.ascontiguousarray(np.asarray(basket_lens)).astype(np.int32)
    slens = np.ascontiguousarray(np.asarray(seq_lens)).astype(np.int32)

    B, s_dim, m_dim = ids.shape
    assert B % N_CORES == 0 and s_dim == S and m_dim == M
    ng = B * S // (N_CORES * P)  # 50 groups per core

    # Host-side slot assignment (pure index/layout work): sort ALL baskets
    # globally by effective length (0 for sequence-padded baskets — no
    # gather needed, their output is the pad constant; else basket_len)
    # and deal 128-basket chunks round-robin to the 8 cores. Group g then
    # needs only L_g = max(eff len in chunk row g) gather instructions,
    # identical on every core (perfectly balanced SPMD program).
    valid = np.arange(S)[None, :] < slens[:, None]            # [B, S]
    eff = np.where(valid, lens, 0).reshape(-1)                # [B*S]
    order = np.argsort(-eff, kind="stable")                   # slot -> basket
    fb, fs = order // S, order % S
    ids_g = ids[fb, fs]                                       # [B*S, M]
    ids_g = np.where(valid[fb, fs][:, None], ids_g, PAD_ID).astype(np.int32)
    lens_g = lens[fb, fs].astype(np.int32)
    sidx_g = fs.astype(np.int32)
    slen_g = slens[fb].astype(np.int32)
    eff_srt = eff[order]

    # slot rank i -> chunk k = i//P (core k%8, group k//8), partition i%P
    def core_view(x):
        # [B*S, ...] slot-ranked -> per-core [P, ng * tail] partition-major
        y = x.reshape(ng, N_CORES, P, -1)                     # [g, c, p, t]
        return [np.ascontiguousarray(
            y[:, c].transpose(1, 0, 2).reshape(P, -1)) for c in range(N_CORES)]

    ids_pc = core_view(ids_g)
    lens_pc = core_view(lens_g)
    sidx_pc = core_view(sidx_g)
    slen_pc = core_view(slen_g)
    lprofile = tuple(int(x) for x in
                     eff_srt.reshape(ng, N_CORES * P).max(axis=1))

    key = (lprofile, ng)
    if key not in _NC_CACHE:
        _NC_CACHE.clear()
        _NC_CACHE[key] = build_nc(lprofile, ng)
    nc = _NC_CACHE[key]

    in_maps = [{"table": table, "ids": ids_pc[c], "lens": lens_pc[c],
                "sidx": sidx_pc[c], "slen": slen_pc[c]}
               for c in range(N_CORES)]
    res = run_bass_kernel_spmd(nc, in_maps, list(range(N_CORES)))

    # res[c]["out"][p, g] holds the basket at global slot rank
    # (g*N_CORES + c)*P + p; invert the layout permutation.
    slot_vals = np.empty((ng, N_CORES, P, H), np.float32)
    for c in range(N_CORES):
        slot_vals[:, c] = res.results[c]["out"].transpose(1, 0, 2)
    out_flat = np.empty((B * S, H), np.float32)
    out_flat[order] = slot_vals.reshape(B * S, H)
    return out_flat.reshape(B, S, H)



# revision 5
# speedup vs baseline: 1.1840x; 1.1840x over previous
"""BasketEmbedding Trainium2 kernel (Bass/Tile, 8 NeuronCores, SPMD).

Reference semantics (B=1024, S=50, M=20, H=128, table 100001x128 f32,
padding_idx = 100000 whose row is zero):

    emb    = table[item_ids]                             # [B,S,M,H]
    summed = sum over m < basket_lens[b,s] of emb        # [B,S,H]
    pooled = summed / basket_lens                        # mean pool
    out    = where(s < seq_lens[b], pooled, 100000.0)    # [B,S,H]

Strategy: data-parallel over batch — each of the 8 cores handles 128
batches (6400 baskets). The dynamic-DMA ucode on this runtime consumes
exactly one offset per contiguous output run per partition, so one
indirect DMA can gather at most 128 table rows (one per partition) and
its ~1.5 us fixed SWDGE cost dominates. To minimize instruction count,
the host assigns baskets to (partition, group) slots sorted by
"effective length" (1 for sequence-padded baskets — their output is a
constant and their single slot points at the zero padding row, else
basket_len), so group g only needs L_g = max length in that group
gather instructions (~300 total instead of 50*20). On device: item
slots past each basket's length are remapped to the zero padding row,
a DVE tensor_reduce sums each group's gathered rows, and a fused
tensor_scalar applies 1/len and the sequence-validity mask. The host
maps output rows back to their natural (b, s) positions (pure layout).
"""

import numpy as np

import concourse.bass as bass
import concourse.mybir as mybir
import concourse.tile as tile
from concourse.bass_utils import run_bass_kernel_spmd

N_CORES = 8


def _split_multi_waits(nc):
    """Walrus on this stack rejects >1 sync-wait command per instruction
    ("Too many sync wait commands", CoreV3GenImpl setupSyncWait). Tile
    freely attaches several SyncWaits to one instruction, so hoist all
    but the last wait of each instruction onto same-engine NoOps
    inserted directly before it — identical sequencer semantics.
    """
    fn = nc.m.functions[0]
    for bb in fn.blocks:
        insts = bb.instructions
        if not any(i.sync_info and i.sync_info.on_wait
                   and len(i.sync_info.on_wait) > 1 for i in insts):
            continue
        new_list = []
        for inst in insts:
            si = inst.sync_info
            if si is not None and si.on_wait and len(si.on_wait) > 1:
                waits = list(si.on_wait)
                for k, w in enumerate(waits[:-1]):
                    nop = mybir.InstNoOp(name=f"{inst.name}-w{k}", ins=[],
                                         outs=[])
                    nop.engine = inst.engine
                    nop.sync_info = mybir.SyncInfo(on_wait=[w], on_update=[])
                    new_list.append(nop)
                inst.sync_info = mybir.SyncInfo(
                    on_wait=[waits[-1]],
                    on_update=list(si.on_update) if si.on_update else [])
            new_list.append(inst)
        bb.instructions = new_list


P = 128        # SBUF partitions = baskets per group; batches per core
S = 50         # sequence positions; also groups per core (6400/128)
M = 20         # max items per basket
H = 128        # hidden size
NROWS = 100001
PAD_ID = 100000
PAD_VAL = 100000.0

F32 = mybir.dt.float32
BF16 = mybir.dt.bfloat16
I32 = mybir.dt.int32
OP = mybir.AluOpType


def build_nc(lprofile, ng, m=M, h=H, nrows=NROWS, pad_id=PAD_ID,
             pad_val=PAD_VAL, gather_bufs=20):
    """Build the per-core program. lprofile[g] = number of item slots to
    gather for group g (= max effective basket length in the group)."""
    nc = bass.Bass()

    table = nc.dram_tensor("table", [nrows, h], BF16, kind="ExternalInput").ap()
    ids = nc.dram_tensor("ids", [P, ng * m], I32, kind="ExternalInput").ap()
    lens = nc.dram_tensor("lens", [P, ng], I32, kind="ExternalInput").ap()
    sidx = nc.dram_tensor("sidx", [P, ng], I32, kind="ExternalInput").ap()
    slen = nc.dram_tensor("slen", [P, ng], I32, kind="ExternalInput").ap()
    out = nc.dram_tensor("out", [P, ng, h], F32, kind="ExternalOutput").ap()

    with tile.TileContext(nc) as tc:
        with (
            tc.tile_pool(name="const", bufs=1) as cpool,
            tc.tile_pool(name="gather", bufs=gather_bufs) as gpool,
            tc.tile_pool(name="acc", bufs=8) as apool,
            tc.tile_pool(name="fin", bufs=8) as fpool,
        ):
            # Tile dependency tracking is tile-granular, so group 0's inputs
            # get physically separate tiles — its gathers then wait only on
            # three tiny ops instead of the full-tensor mask chain.
            ids0_t = cpool.tile([P, m], I32, tag="ids0")
            nc.sync.dma_start(ids0_t[:], ids[:, 0:m])
            lens0_t = cpool.tile([P, 1], I32, tag="lens0")
            nc.sync.dma_start(lens0_t[:], lens[:, 0:1])
            ids_t = cpool.tile([P, ng * m], I32, tag="ids")
            nc.sync.dma_start(ids_t[:, m:], ids[:, m:])
            lens_t = cpool.tile([P, ng], I32, tag="lens")
            nc.sync.dma_start(lens_t[:], lens)
            sidx_t = cpool.tile([P, ng], I32, tag="sidx")
            nc.sync.dma_start(sidx_t[:], sidx)
            slen_t = cpool.tile([P, ng], I32, tag="slen")
            nc.sync.dma_start(slen_t[:], slen)

            # miota[p, g*m + j] = j  (item slot index within basket)
            miota = cpool.tile([P, ng * m], I32, tag="miota")
            nc.gpsimd.iota(miota[:], pattern=[[0, ng], [1, m]], base=0,
                           channel_multiplier=0)

            # Masked ids: slots at/past the basket length -> padding row
            # (whose embedding is all zeros):  id' = max(id, (j>=len)*pad).
            # Computed in two chunks so the first gather group's columns
            # are ready without waiting for the whole id tensor.
            pm0 = cpool.tile([P, m], I32, tag="pm0")
            mid0_t = cpool.tile([P, m], I32, tag="mid0")
            nc.vector.tensor_tensor(
                out=pm0[:], in0=miota[:, 0:m],
                in1=lens0_t[:].to_broadcast([P, m]), op=OP.is_ge)
            nc.vector.tensor_scalar(
                out=pm0[:], in0=pm0[:], scalar1=pad_id, scalar2=None,
                op0=OP.mult)
            nc.vector.tensor_tensor(
                out=mid0_t[:], in0=ids0_t[:], in1=pm0[:], op=OP.max)

            pm = cpool.tile([P, ng * m], I32, tag="pm")
            mid_t = cpool.tile([P, ng * m], I32, tag="mid")
            nc.vector.tensor_tensor(
                out=pm[:, m:], in0=miota[:, m:],
                in1=lens_t[:, 1:ng].broadcast_to([P, ng - 1, m]), op=OP.is_ge)
            nc.vector.tensor_scalar(
                out=pm[:, m:], in0=pm[:, m:], scalar1=pad_id, scalar2=None,
                op0=OP.mult)
            nc.vector.tensor_tensor(
                out=mid_t[:, m:], in0=ids_t[:, m:], in1=pm[:, m:], op=OP.max)

            # Fused epilogue coefficients per slot:
            #   valid  (s <  seq_len): out = acc * (1/len) + 0
            #   padded (s >= seq_len): out = acc * 0       + pad_val
            lens_f = cpool.tile([P, ng], F32, tag="lensf")
            nc.vector.tensor_copy(out=lens_f[:], in_=lens_t[:])
            recip = cpool.tile([P, ng], F32, tag="recip")
            nc.vector.reciprocal(recip[:], lens_f[:])
            smask = cpool.tile([P, ng], F32, tag="smask")
            nc.vector.tensor_tensor(
                out=smask[:], in0=sidx_t[:], in1=slen_t[:], op=OP.is_lt)
            scale = cpool.tile([P, ng], F32, tag="scale")
            nc.vector.tensor_tensor(
                out=scale[:], in0=smask[:], in1=recip[:], op=OP.mult)
            offs = cpool.tile([P, ng], F32, tag="offs")
            nc.vector.tensor_scalar(
                out=offs[:], in0=smask[:], scalar1=-pad_val, scalar2=pad_val,
                op0=OP.mult, op1=OP.add)

            # Emit the all-padded (no-gather) groups first so their copies
            # and stores overlap the gather phase instead of trailing it.
            gorder = ([g for g in range(ng) if lprofile[g] == 0]
                      + [g for g in range(ng) if lprofile[g] > 0])
            for g in gorder:
                lg = int(lprofile[g])
                ft = fpool.tile([P, h], F32, tag="ft")
                if lg == 0:
                    # Group of sequence-padded baskets only: output is the
                    # constant pad vector; no gather needed.
                    nc.vector.tensor_copy(
                        out=ft[:], in_=offs[:, g:g + 1].to_broadcast([P, h]))
                    nc.sync.dma_start(out[:, g, :], ft[:])
                else:
                    gt = gpool.tile([P, lg * h], BF16, tag="gt")
                    # One [P,1]-offset indirect DMA per item slot: the ucode
                    # consumes one offset per contiguous output run/partition.
                    midsrc = mid0_t if g == 0 else mid_t
                    for j in range(lg):
                        nc.gpsimd.indirect_dma_start(
                            out=gt[:, j * h:(j + 1) * h], out_offset=None,
                            in_=table,
                            in_offset=bass.IndirectOffsetOnAxis(
                                ap=midsrc[:, g * m + j:g * m + j + 1], axis=0),
                        )

                    acc = apool.tile([P, h], F32, tag="acc")
                    nc.vector.tensor_reduce(
                        out=acc[:],
                        in_=gt[:].rearrange("p (m h) -> p h m", m=lg),
                        axis=mybir.AxisListType.X, op=OP.add)
                    nc.vector.tensor_scalar(
                        out=ft[:], in0=acc[:],
                        scalar1=scale[:, g:g + 1], scalar2=offs[:, g:g + 1],
                        op0=OP.mult, op1=OP.add)
                    nc.sync.dma_start(out[:, g, :], ft[:])

    _split_multi_waits(nc)
    return nc


_NC_CACHE = {}


def kernel(table, item_ids, basket_lens, seq_lens):
    import ml_dtypes
    table = np.ascontiguousarray(
        np.asarray(table, dtype=np.float32).astype(ml_dtypes.bfloat16))
    ids = np.ascontiguousarray(np.asarray(item_ids)).astype(np.int32)
    lens = np.ascontiguousarray(np.asarray(basket_lens)).astype(np.int32)
    slens = np.ascontiguousarray(np.asarray(seq_lens)).astype(np.int32)

    B, s_dim, m_dim = ids.shape
    assert B % N_CORES == 0 and s_dim == S and m_dim == M
    ng = B * S // (N_CORES * P)  # 50 groups per core

    # Host-side slot assignment (pure index/layout work): sort ALL baskets
    # globally by effective length (0 for sequence-padded baskets — no
    # gather needed, their output is the pad constant; else basket_len)
    # and deal 128-basket chunks round-robin to the 8 cores. Group g then
    # needs only L_g = max(eff len in chunk row g) gather instructions,
    # identical on every core (perfectly balanced SPMD program).
    valid = np.arange(S)[None, :] < slens[:, None]            # [B, S]
    eff = np.where(valid, lens, 0).reshape(-1)                # [B*S]
    order = np.argsort(-eff, kind="stable")                   # slot -> basket
    fb, fs = order // S, order % S
    ids_g = ids[fb, fs]                                       # [B*S, M]
    ids_g = np.where(valid[fb, fs][:, None], ids_g, PAD_ID).astype(np.int32)
    lens_g = lens[fb, fs].astype(np.int32)
    sidx_g = fs.astype(np.int32)
    slen_g = slens[fb].astype(np.int32)
    eff_srt = eff[order]

    # slot rank i -> chunk k = i//P (core k%8, group k//8), partition i%P
    def core_view(x):
        # [B*S, ...] slot-ranked -> per-core [P, ng * tail] partition-major
        y = x.reshape(ng, N_CORES, P, -1)                     # [g, c, p, t]
        return [np.ascontiguousarray(
            y[:, c].transpose(1, 0, 2).reshape(P, -1)) for c in range(N_CORES)]

    ids_pc = core_view(ids_g)
    lens_pc = core_view(lens_g)
    sidx_pc = core_view(sidx_g)
    slen_pc = core_view(slen_g)
    lprofile = tuple(int(x) for x in
                     eff_srt.reshape(ng, N_CORES * P).max(axis=1))

    key = (lprofile, ng)
    if key not in _NC_CACHE:
        _NC_CACHE.clear()
        _NC_CACHE[key] = build_nc(lprofile, ng)
    nc = _NC_CACHE[key]

    in_maps = [{"table": table, "ids": ids_pc[c], "lens": lens_pc[c],
                "sidx": sidx_pc[c], "slen": slen_pc[c]}
               for c in range(N_CORES)]
    res = run_bass_kernel_spmd(nc, in_maps, list(range(N_CORES)))

    # res[c]["out"][p, g] holds the basket at global slot rank
    # (g*N_CORES + c)*P + p; invert the layout permutation.
    slot_vals = np.empty((ng, N_CORES, P, H), np.float32)
    for c in range(N_CORES):
        slot_vals[:, c] = res.results[c]["out"].transpose(1, 0, 2)
    out_flat = np.empty((B * S, H), np.float32)
    out_flat[order] = slot_vals.reshape(B * S, H)
    return out_flat.reshape(B, S, H)

